# revision 7
# baseline (speedup 1.0000x reference)
"""MinGRU layer on 8 Trainium2 NeuronCores (Bass/Tile kernel).

Algorithm
---------
The reference computes, per batch b:
    h_bar = PL_h(x);  z = sigmoid(PL_z(x))          (piecewise-linear maps)
    h[t]  = (1-z[t])*h[t-1] + z[t]*h_bar[t]         (linear recurrence over T)
    out   = h / (max_s |h| + eps)                   (row max-abs normalization)

The piecewise-linear maps (searchsorted + lerp in the reference) are rewritten
as dense matmuls over a "clamp basis":
    PL(x)[n,o] = sum_i w[i,0,o] + sum_{p=1..7} (w[i,p,o]-w[i,p-1,o]) * c_p
    c_p = clamp((x[n,i] - pt[i,p-1]) / (pt[i,p] - pt[i,p-1]), 0, 1)
with the first/last clamp left open on one side so linear extrapolation
matches the reference exactly (c_1 = min(r_1, 1), c_7 = max(r_7, 0)).

Features c_2..c_6 are bounded in [0,1] -> bf16 matmuls.  c_1/c_7 can be huge
(extrapolation through tiny knot gaps) -> float32r (TF32-like) matmuls, which
run at bf16 speed for moving free dim >= 256.

Sharding: core = (batch, output-channel half): 4 batches x 2 halves = 8 cores.
Each core computes all features for its batch (full T), matmuls for its 256
output channels in [o, t] layout, runs the full-T recurrence with the HW
tensor_tensor_scan (state = a*state + b along the free axis), transposes back
to [t, s] via the PE, and normalizes.  The cross-half max needed by the
normalization is a [128 x 16] AllReduce(max) between core pairs.
"""

import sys
sys.path.insert(0, '/opt/trn_rl_repo')

import numpy as np
import ml_dtypes

import concourse.bacc as bacc
import concourse.tile as tile
import concourse.mybir as mybir
from concourse.bass_utils import run_bass_kernel_spmd
from concourse.masks import make_identity

F32 = mybir.dt.float32
F32R = mybir.dt.float32r
BF16 = mybir.dt.bfloat16
AOT = mybir.AluOpType
AFT = mybir.ActivationFunctionType

B, T, I, S, NKNOT = 4, 2048, 512, 512, 8
EPS = 1e-6
NCORES = 8
OH = S // 2            # output channels per core
N_IT = I // 128        # 4 i-tiles
N_OT = OH // 128       # 2 o-tiles
CHUNK = 512
N_CH = T // CHUNK      # 4 chunks
GRP = 1024             # feature-group width for bounded (bf16) features
N_GR = T // GRP        # 2 groups
CPG = N_CH // N_GR     # chunks per group
N_TB = T // 128        # 16 t-blocks

NF = 7                 # features per input channel
BOUNDED = (1, 2, 3, 4, 5)   # feature indices with clamp01 -> bf16
UNBOUNDED = (0, 6)          # open-ended clamps -> f32r

# engine assignment for bounded features: (affine_engine, clamp_engine)
# 'A' = ACT relu-affine (then clamp stage is just min1); 'V' = DVE; 'G' = GPSIMD
FEAT_ENG = {
    1: ('V', 'V'),
    2: ('G', 'G'),
    3: ('A', 'G'),
    4: ('A', 'V'),
    5: ('A', 'V'),
}

NORM_MODE = "allreduce"   # "allreduce" | "twopass1" (debug: unnormalized out)


def build_kernel():
    nc = bacc.Bacc("TRN2", target_bir_lowering=False, num_devices=NCORES)

    xT_d = nc.dram_tensor("xT", [I, T], F32, kind="ExternalInput")
    wb_d = nc.dram_tensor("wb", [N_IT, 128, 2 * 5 * N_OT * 128], BF16,
                          kind="ExternalInput")
    wr_d = nc.dram_tensor("wr", [N_IT, 128, 2 * 2 * N_OT * 128], F32R,
                          kind="ExternalInput")
    fs_d = nc.dram_tensor("fs", [128, 2 * N_IT, NF], F32, kind="ExternalInput")
    fb_d = nc.dram_tensor("fb", [128, 2 * N_IT, NF], F32, kind="ExternalInput")
    # per-o-tile vectors [o_local, kind, ot]: kind 0=bz 1=-bz 2=bh 3=h0
    vec_d = nc.dram_tensor("vec", [128, 4, N_OT], F32, kind="ExternalInput")

    out_d = nc.dram_tensor("out", [T, OH], F32, kind="ExternalOutput")
    mpart_d = nc.dram_tensor("mpart", [128, N_TB], F32, kind="ExternalOutput")

    zacc, hacc, hsc, ht_sb = {}, {}, {}, {}

    with tile.TileContext(nc) as tc:
        with (
            tc.tile_pool(name="const", bufs=1) as constp,
            tc.tile_pool(name="wbp", bufs=N_IT) as wbp,
            tc.tile_pool(name="wrp", bufs=N_IT) as wrp,
            tc.tile_pool(name="xtp", bufs=6) as xtp,
            tc.tile_pool(name="rtp", bufs=4) as rtp,
            tc.tile_pool(name="featb", bufs=8) as featbp,
            tc.tile_pool(name="featr", bufs=5) as featrp,
            tc.tile_pool(name="scant", bufs=2) as scantp,
            tc.tile_pool(name="hscp", bufs=8) as hscp,
            tc.tile_pool(name="htp", bufs=N_TB) as htp,
            tc.tile_pool(name="zps", bufs=2 * CPG, space="PSUM") as zpsp,
            tc.tile_pool(name="hps", bufs=2 * CPG, space="PSUM") as hpsp,
            tc.tile_pool(name="dram", bufs=1, space="DRAM") as dramp,
        ):
            # ---- constants ----
            identf = constp.tile([128, 128], F32)
            make_identity(nc, identf)
            fs_t = constp.tile([128, 2 * N_IT, NF], F32)
            nc.sync.dma_start(fs_t[:], fs_d[:])
            fb_t = constp.tile([128, 2 * N_IT, NF], F32)
            nc.sync.dma_start(fb_t[:], fb_d[:])
            vec_t = constp.tile([128, 4, N_OT], F32)
            nc.sync.dma_start(vec_t[:], vec_d[:])
            m_all = constp.tile([128, N_TB], F32)
            minv = constp.tile([128, N_TB], F32)

            # ---- prefetch first feature group's x columns ----
            xt_pre = []
            for it in range(N_IT):
                xt0 = xtp.tile([128, GRP], F32, tag="xt", name=f"xtpre{it}")
                nc.sync.dma_start(xt0[:], xT_d[it * 128:(it + 1) * 128, 0:GRP])
                xt_pre.append(xt0)

            # ---- weights (resident); f32r first (consumed by first k-tiles)
            wb_t, wr_t = [], []
            for it in range(N_IT):
                tr = wrp.tile([128, 2 * 2 * N_OT * 128], F32R, tag="wr")
                nc.sync.dma_start(tr[:], wr_d[it])
                wr_t.append(tr)
            for it in range(N_IT):
                tb = wbp.tile([128, 2 * 5 * N_OT * 128], BF16, tag="wb")
                nc.sync.dma_start(tb[:], wb_d[it])
                wb_t.append(tb)

            def wb_slice(path, fbi, it, ot):
                k = (path * 5 + fbi) * N_OT + ot
                return wb_t[it][:, k * 128:(k + 1) * 128]

            def wr_slice(path, fui, it, ot):
                k = (path * 2 + fui) * N_OT + ot
                return wr_t[it][:, k * 128:(k + 1) * 128]

            # feature order per path: interleave producer engines
            FEAT_ORDER = (6, 1, 3, 2, 4, 0, 5)
            NKT = NF * N_IT            # k-tiles per path

            for g in range(N_GR):
                if g == 0:
                    xt_g = xt_pre
                else:
                    xt_g = []
                    for it in range(N_IT):
                        xt = xtp.tile([128, GRP], F32, tag="xt")
                        nc.sync.dma_start(
                            xt[:], xT_d[it * 128:(it + 1) * 128,
                                        g * GRP:(g + 1) * GRP])
                        xt_g.append(xt)

                for path in range(2):
                    # allocate this path's accumulators (2 o-tiles x 2 chunks)
                    accs = {}
                    for cl in range(CPG):
                        for ot in range(N_OT):
                            pool = zpsp if path == 0 else hpsp
                            accs[(ot, cl)] = pool.tile(
                                [128, CHUNK], F32,
                                name=f"acc{path}_{ot}_{cl}",
                                tag="zacc" if path == 0 else "hacc")

                    # k-outer: produce one feature tile, immediately consume
                    # it with its 4 matmuls (2 o-tiles x 2 chunks)
                    ki = 0
                    for f in FEAT_ORDER:
                        for it in range(N_IT):
                            fsl = fs_t[:, path * N_IT + it, f:f + 1]
                            fbl = fb_t[:, path * N_IT + it, f:f + 1]
                            if f in UNBOUNDED:
                                ct = featrp.tile([128, GRP], F32R,
                                                 tag="featr")
                                if f == 6:
                                    nc.scalar.activation(ct, xt_g[it],
                                                         AFT.Relu,
                                                         bias=fbl, scale=fsl)
                                else:
                                    rt = rtp.tile([128, GRP], F32, tag="rt")
                                    nc.vector.tensor_scalar(
                                        rt, xt_g[it], fsl, fbl,
                                        op0=AOT.mult, op1=AOT.add)
                                    nc.vector.tensor_scalar(
                                        ct, rt, 1.0, None, op0=AOT.min)
                                wsl = lambda ot, f=f: wr_slice(
                                    path, 0 if f == 0 else 1, it, ot)
                            else:
                                ae, ce = FEAT_ENG[f]
                                ct = featbp.tile([128, GRP], BF16,
                                                 tag="featb")
                                if ae == 'A':
                                    rt = rtp.tile([128, GRP], F32, tag="rt")
                                    nc.scalar.activation(rt, xt_g[it],
                                                         AFT.Relu,
                                                         bias=fbl, scale=fsl)
                                    eng = (nc.vector if ce == 'V'
                                           else nc.gpsimd)
                                    eng.tensor_scalar(ct, rt, 1.0, None,
                                                      op0=AOT.min)
                                else:
                                    eng = (nc.vector if ae == 'V'
                                           else nc.gpsimd)
                                    rt = rtp.tile([128, GRP], F32, tag="rt")
                                    eng.tensor_scalar(rt, xt_g[it], fsl, fbl,
                                                      op0=AOT.mult,
                                                      op1=AOT.add)
                                    eng2 = (nc.vector if ce == 'V'
                                            else nc.gpsimd)
                                    eng2.tensor_scalar(ct, rt, 0.0, 1.0,
                                                       op0=AOT.max,
                                                       op1=AOT.min)
                                wsl = lambda ot, f=f: wb_slice(
                                    path, BOUNDED.index(f), it, ot)

                            first = ki == 0
                            last = ki == NKT - 1
                            for cl in range(CPG):
                                for ot in range(N_OT):
                                    nc.tensor.matmul(
                                        accs[(ot, cl)], wsl(ot),
                                        ct[:, cl * CHUNK:(cl + 1) * CHUNK],
                                        start=first, stop=last,
                                        skip_group_check=True)
                            ki += 1

                    for cl in range(CPG):
                        c = g * CPG + cl
                        for ot in range(N_OT):
                            (zacc if path == 0 else hacc)[(ot, c)] = \
                                accs[(ot, cl)]

                # scans + transposes for this group's chunks
                for cl in range(CPG):
                    c = g * CPG + cl
                    for ot in range(N_OT):
                        zps = zacc.pop((ot, c))
                        hps = hacc.pop((ot, c))
                        zt = scantp.tile([128, CHUNK], F32, tag="zt")
                        nc.scalar.activation(zt, zps, AFT.Sigmoid,
                                             bias=vec_t[:, 0, ot:ot + 1],
                                             scale=1.0)
                        at = scantp.tile([128, CHUNK], F32, tag="at")
                        nc.scalar.activation(at, zps, AFT.Sigmoid,
                                             bias=vec_t[:, 1, ot:ot + 1],
                                             scale=-1.0)
                        bt = scantp.tile([128, CHUNK], F32, tag="bt")
                        nc.vector.scalar_tensor_tensor(
                            bt, hps, vec_t[:, 2, ot:ot + 1], zt,
                            op0=AOT.add, op1=AOT.mult)
                        hh = hscp.tile([128, CHUNK], F32, tag="hsc")
                        init = (vec_t[:, 3, ot:ot + 1] if c == 0
                                else hsc[(ot, c - 1)][:, CHUNK - 1:CHUNK])
                        nc.vector.tensor_tensor_scan(
                            hh, at, bt, init, op0=AOT.mult, op1=AOT.add)
                        hsc[(ot, c)] = hh

                    for tb in range(CHUNK // 128):
                        g8 = c * (CHUNK // 128) + tb
                        tps = hpsp.tile([128, N_OT * 128], F32, tag="hacc")
                        for ot in range(N_OT):
                            nc.tensor.transpose(
                                tps[:, ot * 128:(ot + 1) * 128],
                                hsc[(ot, c)][:, tb * 128:(tb + 1) * 128],
                                identf)
                        nc.vector.tensor_reduce(
                            m_all[:, g8:g8 + 1], tps,
                            axis=mybir.AxisListType.X, op=AOT.max,
                            apply_absolute_value=True)
                        ht = htp.tile([128, OH], F32, tag="ht")
                        nc.scalar.copy(ht, tps)
                        ht_sb[g8] = ht

            # ---- normalization finale ----
            nc.sync.dma_start(mpart_d[:], m_all[:])
            if NORM_MODE == "allreduce":
                cin = dramp.tile([128, N_TB], F32)
                cout = dramp.tile([128, N_TB], F32)
                nc.sync.dma_start(cin[:], m_all[:])
                nc.gpsimd.collective_compute(
                    "AllReduce", AOT.max,
                    replica_groups=[[0, 1], [2, 3], [4, 5], [6, 7]],
                    ins=[cin.opt()], outs=[cout.opt()])
                m_red = constp.tile([128, N_TB], F32)
                nc.sync.dma_start(m_red[:], cout[:])
                nc.vector.tensor_scalar(minv, m_red, float(EPS), None,
                                        op0=AOT.add)
                nc.vector.reciprocal(minv, minv)
                for g8 in range(N_TB):
                    ht = ht_sb[g8]
                    nc.vector.tensor_scalar(ht, ht, minv[:, g8:g8 + 1], None,
                                            op0=AOT.mult)
                    nc.sync.dma_start(out_d[g8 * 128:(g8 + 1) * 128, :], ht[:])
            else:
                for g8 in range(N_TB):
                    nc.sync.dma_start(out_d[g8 * 128:(g8 + 1) * 128, :],
                                      ht_sb[g8][:])

    nc.compile()
    return nc


_NC_CACHE = None


def _get_nc():
    global _NC_CACHE
    if _NC_CACHE is None:
        _NC_CACHE = build_kernel()
    return _NC_CACHE


def _prep_core_inputs(x, h, z_pts, z_w, h_pts, h_w):
    f32 = np.float32
    paths = []
    for pts, w in ((z_pts, z_w), (h_pts, h_w)):
        g = np.diff(pts.astype(f32), axis=1)              # [I,7]
        dw = np.diff(w.astype(f32), axis=1)               # [I,7,S]
        bias = w[:, 0, :].astype(np.float64).sum(axis=0).astype(f32)  # [S]
        inv = (1.0 / g).astype(f32)
        fb = (-pts[:, :-1].astype(f32) * inv).astype(f32)
        paths.append((inv, fb, dw, bias))

    fs_host = np.zeros((128, 2 * N_IT, NF), f32)
    fb_host = np.zeros((128, 2 * N_IT, NF), f32)
    for path in range(2):
        inv, fbv = paths[path][0], paths[path][1]
        for it in range(N_IT):
            fs_host[:, path * N_IT + it, :] = inv[it * 128:(it + 1) * 128, :]
            fb_host[:, path * N_IT + it, :] = fbv[it * 128:(it + 1) * 128, :]

    in_maps = []
    for core in range(NCORES):
        b, half = divmod(core, 2)
        osl = slice(half * OH, (half + 1) * OH)
        xT = np.ascontiguousarray(x[b].T.astype(f32))     # [I, T]

        wb = np.zeros((N_IT, 128, 2 * 5 * N_OT * 128), ml_dtypes.bfloat16)
        wr = np.zeros((N_IT, 128, 2 * 2 * N_OT * 128), f32)
        for path in range(2):
            dw = paths[path][2][:, :, osl]                # [I, 7, OH]
            for it in range(N_IT):
                dwi = dw[it * 128:(it + 1) * 128]         # [128, 7, OH]
                for fbi, f in enumerate(BOUNDED):
                    for ot in range(N_OT):
                        k = (path * 5 + fbi) * N_OT + ot
                        wb[it, :, k * 128:(k + 1) * 128] = \
                            dwi[:, f, ot * 128:(ot + 1) * 128].astype(
                                ml_dtypes.bfloat16)
                for fui, f in enumerate(UNBOUNDED):
                    for ot in range(N_OT):
                        k = (path * 2 + fui) * N_OT + ot
                        wr[it, :, k * 128:(k + 1) * 128] = \
                            dwi[:, f, ot * 128:(ot + 1) * 128]

        vec = np.zeros((128, 4, N_OT), f32)
        for ot in range(N_OT):
            o0 = half * OH + ot * 128
            bz = paths[0][3][o0:o0 + 128]
            bh = paths[1][3][o0:o0 + 128]
            vec[:, 0, ot] = bz
            vec[:, 1, ot] = -bz
            vec[:, 2, ot] = bh
            vec[:, 3, ot] = h[b, o0:o0 + 128].astype(f32)

        in_maps.append(dict(xT=xT, wb=wb, wr=wr, fs=fs_host, fb=fb_host,
                            vec=vec))
    return in_maps


def kernel(x, h, z_pts, z_w, h_pts, h_w):
    x = np.asarray(x); h = np.asarray(h)
    z_pts = np.asarray(z_pts); z_w = np.asarray(z_w)
    h_pts = np.asarray(h_pts); h_w = np.asarray(h_w)

    nc = _get_nc()
    in_maps = _prep_core_inputs(x, h, z_pts, z_w, h_pts, h_w)
    res = run_bass_kernel_spmd(nc, in_maps, core_ids=list(range(NCORES)))

    out = np.empty((B, T, S), np.float32)
    if NORM_MODE == "allreduce":
        for core in range(NCORES):
            b, half = divmod(core, 2)
            out[b, :, half * OH:(half + 1) * OH] = res.results[core]["out"]
    else:
        for b in range(B):
            m0 = res.results[2 * b]["mpart"]
            m1 = res.results[2 * b + 1]["mpart"]
            m = np.maximum(m0, m1).T.reshape(T, 1)
            out[b, :, :OH] = res.results[2 * b]["out"] / (m + EPS)
            out[b, :, OH:] = res.results[2 * b + 1]["out"] / (m + EPS)
    return out
